# revision 6
# baseline (speedup 1.0000x reference)
"""DenseSSM layer kernel for Trainium2 (8 NeuronCores, data-parallel over batch).

Reference computation per batch row r:
    d  = sigmoid(u @ Wd + bd)                      [T, N]
    A  = tanh(u @ WA + bA).reshape(T,N,N)/sqrt(N)  with diagonal replaced by d
    Bt = u @ WB + bB                               [T, N]
    h_t = A_t h_{t-1} + Bt_t   (sequential scan)
    y  = hs @ C + D_skip * u                       [T, DM]

Kernel strategy (per core; core i handles batch row i % 4, half the T range):
  - Big GEMM u@WA in fp16, WA stationary per (slice s, k-tile); PSUM [m, t]
    evacuated by ACT tanh (bias bA) into a t-major bigbuf[m, t, s] so the
    scan's stationary read bigs[:, t, :] is contiguous (FWL on LDWEIGHTS).
  - Diagonal: WA/bA diag slots zeroed on host (tanh(0)=0); after each slice's
    evac, DVE overwrites row s of slice s with sqrt(N)*d so the scan matvec
    applies the diagonal d*h term together with the off-diag part.  These DVE
    writes interleave with the scan updates and hide in the chain's DVE idle.
  - Scan (the critical serial chain): per step, PE matvec p = BIGS_t^T h, then
    a single DVE op h_{t+1} = p*ISN + Bt (tensor_scalar, PSUM src).  Scan
    steps for chunk c-1 interleave into chunk c's GEMM instruction stream.
  - y GEMM per 128-timestep block from h_sb against C (fp16) + D_skip*u on DVE.
  - WA streamed per chunk (4 chunks of 272) on both HWDGE rings (sync+scalar).
"""

import sys

sys.path.insert(0, "/opt/trn_rl_repo")

import numpy as np
from contextlib import ExitStack

import concourse.bass as bass
import concourse.tile as tile
from concourse import bacc, mybir
from concourse.bass_utils import run_bass_kernel_spmd

F16 = mybir.dt.float16
F32 = mybir.dt.float32
AFT = mybir.ActivationFunctionType

B, T, DM, N = 4, 2048, 1024, 128
KT = DM // 128          # 8 contraction tiles
SQN = float(np.sqrt(N))
ISN = float(1.0 / np.sqrt(N))


def build_nc(t_total, chunks):
    assert sum(chunks) == t_total
    nchunks = len(chunks)
    offs = [0]
    for w in chunks:
        offs.append(offs[-1] + w)
    nc = bacc.Bacc("TRN2", debug=False)

    uT = nc.dram_tensor("uT", [DM, t_total], F16, kind="ExternalInput").ap()
    u16 = nc.dram_tensor("u16", [t_total, DM], F16, kind="ExternalInput").ap()
    WAh = nc.dram_tensor("WAh", [N, 128, KT * 128], F16, kind="ExternalInput").ap()
    Wdh = nc.dram_tensor("Wdh", [128, KT * N], F16, kind="ExternalInput").ap()
    WBh = nc.dram_tensor("WBh", [128, KT * N], F16, kind="ExternalInput").ap()
    bAb = nc.dram_tensor("bAb", [N, N], F32, kind="ExternalInput").ap()
    bdv = nc.dram_tensor("bdv", [N, 1], F32, kind="ExternalInput").ap()
    bBv = nc.dram_tensor("bBv", [N, 1], F32, kind="ExternalInput").ap()
    Cw = nc.dram_tensor("Cw", [N, DM], F16, kind="ExternalInput").ap()
    Dfl = nc.dram_tensor("Dfl", [128, DM], F32, kind="ExternalInput").ap()
    yout_d = nc.dram_tensor("y", [t_total, DM], F32, kind="ExternalOutput").ap()

    with tile.TileContext(nc) as tc:
        with ExitStack() as ctx:
            cpool = ctx.enter_context(tc.tile_pool(name="consts", bufs=1))
            wa_pool = ctx.enter_context(tc.tile_pool(name="wa", bufs=3))
            ut_pool = ctx.enter_context(tc.tile_pool(name="ut", bufs=2))
            big_pool = ctx.enter_context(tc.tile_pool(name="big", bufs=2))
            h_pool = ctx.enter_context(tc.tile_pool(name="h", bufs=1))
            d_pool = ctx.enter_context(tc.tile_pool(name="d", bufs=2))
            b_pool = ctx.enter_context(tc.tile_pool(name="bt", bufs=2))
            u16_pool = ctx.enter_context(tc.tile_pool(name="u16t", bufs=2))
            yo_pool = ctx.enter_context(tc.tile_pool(name="yo", bufs=2))
            ty_pool = ctx.enter_context(tc.tile_pool(name="ty", bufs=2))
            psg = ctx.enter_context(tc.tile_pool(name="psg", bufs=2, space="PSUM"))
            pss = ctx.enter_context(tc.tile_pool(name="pss", bufs=2, space="PSUM"))
            psp = ctx.enter_context(tc.tile_pool(name="psp", bufs=2, space="PSUM"))

            # ---- constants ----
            wd_sb = cpool.tile([128, KT * N], F16)
            nc.sync.dma_start(wd_sb[:], Wdh)
            wb_sb = cpool.tile([128, KT * N], F16)
            nc.sync.dma_start(wb_sb[:], WBh)
            bab_sb = cpool.tile([N, N], F32)
            nc.sync.dma_start(bab_sb[:], bAb)
            bd_sb = cpool.tile([N, 1], F32)
            nc.sync.dma_start(bd_sb[:], bdv)
            bb_sb = cpool.tile([N, 1], F32)
            nc.sync.dma_start(bb_sb[:], bBv)
            c_sb = cpool.tile([N, DM], F16)
            nc.sync.dma_start(c_sb[:], Cw)
            dfl_sb = cpool.tile([128, DM], F32)
            nc.sync.dma_start(dfl_sb[:], Dfl)

            h_sb = h_pool.tile([128, t_total + 8], F16)
            nc.vector.memset(h_sb[:, 0:1], 0.0)

            bigs = [None, None]
            dsbs = [None, None]
            bsbs = [None, None]

            for c in range(nchunks + 1):
                cw = chunks[c] if c < nchunks else 0
                pw = chunks[c - 1] if c >= 1 else 0   # scan-chunk width
                po = offs[c - 1] if c >= 1 else 0     # scan-chunk offset
                if c < nchunks:
                    t0 = offs[c]
                    ut = ut_pool.tile([128, KT, cw], F16, tag="ut")
                    for k in range(KT):
                        nc.sync.dma_start(
                            ut[:, k, :], uT[k * 128 : (k + 1) * 128, t0 : t0 + cw]
                        )
                    # d = sigmoid(u Wd + bd)
                    pd = pss.tile([128, 512], F32, tag="small")
                    for k in range(KT):
                        nc.tensor.matmul(
                            pd[:, :cw],
                            wd_sb[:, k * N : (k + 1) * N],
                            ut[:, k, :],
                            start=(k == 0),
                            stop=(k == KT - 1),
                        )
                    dsb = d_pool.tile([N, cw], F32, tag="dsb")
                    nc.scalar.activation(
                        dsb[:], pd[:, :cw], AFT.Sigmoid, bias=bd_sb[:, 0:1]
                    )
                    dsbs[c % 2] = dsb
                    # Bt = u WB + bB
                    pb = pss.tile([128, 512], F32, tag="small")
                    for k in range(KT):
                        nc.tensor.matmul(
                            pb[:, :cw],
                            wb_sb[:, k * N : (k + 1) * N],
                            ut[:, k, :],
                            start=(k == 0),
                            stop=(k == KT - 1),
                        )
                    bsb = b_pool.tile([N, cw], F32, tag="bsb")
                    nc.scalar.activation(
                        bsb[:], pb[:, :cw], AFT.Identity, bias=bb_sb[:, 0:1]
                    )
                    bsbs[c % 2] = bsb
                    # pre-scaled fp16 diag source: sqrt(N)*d
                    dsc = d_pool.tile([N, cw], F16, tag="dsc")
                    nc.vector.tensor_scalar(
                        dsc[:], dsb[:], SQN, None, mybir.AluOpType.mult
                    )
                    dscs = dsc

                    # s-major bigbuf: bigs[m, s, t] = A_t[s, m] (pre-diag)
                    bigbuf = big_pool.tile([128, N, cw], F16, tag="bigbuf")
                    bigs[c % 2] = bigbuf

                def scan_step(tl):
                    """One scan timestep of chunk c-1: PE matvec then a single
                    DVE update h_{t+1} = pp*ISN + B_t.  The diagonal d*h term
                    rides inside the matvec (diag-injected bigbuf)."""
                    prev = (c - 1) % 2
                    tg = po + tl
                    pp = psp.tile([128, 1], F32)
                    nc.tensor.matmul(
                        pp[:],
                        bigs[prev][:, :, tl : tl + 1],
                        h_sb[:, tg : tg + 1],
                        start=True,
                        stop=True,
                    )
                    nc.vector.tensor_scalar(
                        h_sb[:, tg + 1 : tg + 2],
                        pp[:],
                        ISN,
                        bsbs[prev][:, tl : tl + 1],
                        mybir.AluOpType.mult,
                        mybir.AluOpType.add,
                    )

                emitted = 0
                y_done = 0

                def emit_scan_to(target):
                    nonlocal emitted
                    while emitted < target:
                        scan_step(emitted)
                        emitted += 1

                def emit_y_ready():
                    # emit y-blocks of chunk c-1 as soon as their scan steps
                    # are complete
                    nonlocal y_done
                    while y_done < pw and y_done + min(128, pw - y_done) <= emitted:
                        tw = min(128, pw - y_done)
                        tstart = po + y_done
                        y_done += tw
                        for dh in range(DM // 512):
                            py = pss.tile([128, 512], F32, tag="small")
                            nc.tensor.matmul(
                                py[:tw, :],
                                h_sb[:, 1 + tstart : 1 + tstart + tw],
                                c_sb[:, dh * 512 : (dh + 1) * 512],
                                start=True,
                                stop=True,
                            )
                            u16t = u16_pool.tile([128, 512], F16)
                            nc.sync.dma_start(
                                u16t[:tw, :],
                                u16[tstart : tstart + tw, dh * 512 : (dh + 1) * 512],
                            )
                            tyt = ty_pool.tile([128, 512], F32)
                            nc.vector.tensor_mul(
                                tyt[:tw, :], u16t[:tw, :],
                                dfl_sb[:tw, dh * 512 : (dh + 1) * 512],
                            )
                            yo = yo_pool.tile([128, 512], F32)
                            nc.vector.tensor_add(yo[:tw, :], py[:tw, :], tyt[:tw, :])
                            nc.sync.dma_start(
                                yout_d[tstart : tstart + tw, dh * 512 : (dh + 1) * 512],
                                yo[:tw, :],
                            )

                WB_BATCH = 4  # slices per WA DMA transfer (1 MiB each)
                for s in range(N):
                    if c < nchunks:
                        if s % WB_BATCH == 0:
                            wa = wa_pool.tile([128, WB_BATCH, KT * 128], F16)
                            # WA gets the scalar HWDGE ring to itself: the sync
                            # ring's diag DMAs wait on ACT evacs and would
                            # head-of-line-block WA prefetch (ring is FIFO).
                            nc.scalar.dma_start(
                                wa[:],
                                WAh[s : s + WB_BATCH].rearrange("s p f -> p s f"),
                            )
                        pg = psg.tile([128, 512], F32, tag="pg")
                        for k in range(KT):
                            nc.tensor.matmul(
                                pg[:, :cw],
                                wa[:, s % WB_BATCH, k * 128 : (k + 1) * 128],
                                ut[:, k, :],
                                start=(k == 0),
                                stop=(k == KT - 1),
                            )
                            if c >= 1 and k in (1, 3, 5):
                                emit_scan_to(
                                    ((s * KT + k + 1) * pw) // (N * KT)
                                )
                        nc.scalar.activation(
                            bigs[c % 2][:, s, :], pg[:, :cw], AFT.Tanh,
                            bias=bab_sb[:, s : s + 1],
                        )
                        # diag injection: row s of slice s <- sqrt(N)*d[s, :]
                        # (single-descriptor SBUF->SBUF DMA; compute engines
                        # cannot address a lone partition at arbitrary base)
                        nc.sync.dma_start(
                            bigs[c % 2][s : s + 1, s, :],
                            dscs[s : s + 1, :],
                        )
                        if c >= 1:
                            emit_scan_to(((s + 1) * pw) // N)
                            emit_y_ready()
                    elif c >= 1:
                        emit_scan_to(((s + 1) * pw) // N)
                        emit_y_ready()

                if c >= 1:
                    emit_y_ready()
                    assert y_done == pw and emitted == pw
    nc.compile()
    return nc


def prep_inputs(u_row, Wd, bd, WA, bA, WB, bB, C, D_skip, t_total=T):
    """Host-side packing of one batch row's inputs into the kernel layout."""
    f16 = np.float16
    idx = np.arange(N)
    WAz = np.array(WA, np.float32, copy=True)
    WAz[:, idx * N + idx] = 0.0
    bAz = np.array(bA, np.float32, copy=True)
    bAz[idx * N + idx] = 0.0
    # WAh[s, p, k*128+m] = WAz[k*128+p, s*N+m]
    WAhost = np.ascontiguousarray(
        WAz.reshape(KT, 128, N, N).transpose(2, 1, 0, 3).reshape(N, 128, KT * 128)
    ).astype(f16)
    Wdh = np.ascontiguousarray(
        np.asarray(Wd, np.float32).reshape(KT, 128, N).transpose(1, 0, 2).reshape(128, KT * N)
    ).astype(f16)
    WBh = np.ascontiguousarray(
        np.asarray(WB, np.float32).reshape(KT, 128, N).transpose(1, 0, 2).reshape(128, KT * N)
    ).astype(f16)
    return {
        "uT": np.ascontiguousarray(u_row.T).astype(f16),
        "u16": np.ascontiguousarray(u_row).astype(f16),
        "WAh": WAhost,
        "Wdh": Wdh,
        "WBh": WBh,
        "bAb": np.ascontiguousarray(bAz.reshape(N, N).T).astype(np.float32),
        "bdv": np.asarray(bd, np.float32).reshape(N, 1).copy(),
        "bBv": np.asarray(bB, np.float32).reshape(N, 1).copy(),
        "Cw": np.asarray(C, np.float32).astype(f16),
        "Dfl": np.ascontiguousarray(
            np.broadcast_to(np.asarray(D_skip, np.float32), (128, DM))
        ).copy(),
    }


_NC_CACHE = {}

# Each batch row r is handled by the core pair (r, r+4): core r covers
# t in [0, 1088), core r+4 covers t in [960, 2048).  Both run the same
# T_LOCAL=1088 program; core r+4's first 128 steps (scanned from h=0) are
# warm-up — the state contracts by ~0.9/step, so by local t=128 the state
# matches the true one to ~1e-6 and its outputs [128:1088) are valid.
T_LOCAL = 1088
CHUNKS_LOCAL = [272, 272, 272, 272]
SHIFT = T - T_LOCAL  # 960
SPLIT = T_LOCAL      # first core's valid range
WARM = 128


def make_in_maps(u, Wd, bd, WA, bA, WB, bB, C, D_skip):
    in_maps = []
    for core in range(8):
        r, half = core % B, core // B
        off = half * SHIFT
        in_maps.append(
            prep_inputs(
                u[r, off : off + T_LOCAL], Wd, bd, WA, bA, WB, bB, C, D_skip,
                t_total=T_LOCAL,
            )
        )
    return in_maps


def kernel(u, Wd, bd, WA, bA, WB, bB, C, D_skip):
    u = np.asarray(u, np.float32)
    if "nc" not in _NC_CACHE:
        _NC_CACHE["nc"] = build_nc(T_LOCAL, CHUNKS_LOCAL)
    nc = _NC_CACHE["nc"]

    in_maps = make_in_maps(u, Wd, bd, WA, bA, WB, bB, C, D_skip)
    res = run_bass_kernel_spmd(nc, in_maps, core_ids=list(range(8)))
    y = np.empty((B, T, DM), np.float32)
    for r in range(B):
        y[r, :SPLIT] = res.results[r]["y"][:SPLIT]
        y[r, SPLIT:] = res.results[r + B]["y"][SPLIT - SHIFT :]
    return y


# revision 12
# speedup vs baseline: 1.4945x; 1.4945x over previous
"""DenseSSM layer kernel for Trainium2 (8 NeuronCores, data-parallel over batch).

Reference computation per batch row r:
    d  = sigmoid(u @ Wd + bd)                      [T, N]
    A  = tanh(u @ WA + bA).reshape(T,N,N)/sqrt(N)  with diagonal replaced by d
    Bt = u @ WB + bB                               [T, N]
    h_t = A_t h_{t-1} + Bt_t   (sequential scan)
    y  = hs @ C + D_skip * u                       [T, DM]

Kernel strategy (per core; core i handles batch row i % 4, half the T range):
  - Big GEMM u@WA in fp16, WA stationary per (slice s, k-tile); PSUM [m, t]
    evacuated by ACT tanh (bias bA) into bigbuf[m, s, t].  WA/bA diag slots
    are zeroed on host so the stored A has zero diagonal.
  - Scan (the critical serial chain): per step, PE matvec p = BIGS_t^T h and
    DVE dhb = d*h + B (both fire when h_{t-1} lands; dhb executes during the
    matvec's drain window), then one DVE update h_t = p*ISN + dhb.  All scan
    vector work stays on DVE: ACT only runs tanh evacs, so no 500ns+ tanh op
    ever head-of-line-blocks a chain update (ACT/DVE queues are strict FIFO).
  - Scan steps for chunk c-1 interleave into chunk c's GEMM instruction
    stream; the whole kernel is a conveyor paced by the serial chain
    (~0.49us/step) with the GEMM (~0.43us/step of PE work) in its shadow.
  - y GEMM per 128-timestep block from h_sb against C (fp16) + D_skip*u on DVE.
  - WA streamed per chunk (4 chunks of 272 -> 128MB) on the sync HWDGE ring
    only: the scalar ring shares the ACT instruction queue and DMAs there
    head-of-line-block the tanh evacs.
"""

import sys

sys.path.insert(0, "/opt/trn_rl_repo")

import numpy as np
from contextlib import ExitStack

import concourse.bass as bass
import concourse.tile as tile
from concourse import bacc, mybir
from concourse.bass_utils import run_bass_kernel_spmd

F16 = mybir.dt.float16
F32 = mybir.dt.float32
AFT = mybir.ActivationFunctionType

B, T, DM, N = 4, 2048, 1024, 128
KT = DM // 128          # 8 contraction tiles
SQN = float(np.sqrt(N))
ISN = float(1.0 / np.sqrt(N))


def build_nc(t_total, chunks):
    assert sum(chunks) == t_total
    nchunks = len(chunks)
    offs = [0]
    for w in chunks:
        offs.append(offs[-1] + w)
    nc = bacc.Bacc("TRN2", debug=False)

    uT = nc.dram_tensor("uT", [DM, t_total], F16, kind="ExternalInput").ap()
    u16 = nc.dram_tensor("u16", [t_total, DM], F16, kind="ExternalInput").ap()
    WAh = nc.dram_tensor("WAh", [N, 128, KT * 128], F16, kind="ExternalInput").ap()
    Wdh = nc.dram_tensor("Wdh", [128, KT * N], F16, kind="ExternalInput").ap()
    WBh = nc.dram_tensor("WBh", [128, KT * N], F16, kind="ExternalInput").ap()
    bAb = nc.dram_tensor("bAb", [N, N], F32, kind="ExternalInput").ap()
    bdv = nc.dram_tensor("bdv", [N, 1], F32, kind="ExternalInput").ap()
    bBv = nc.dram_tensor("bBv", [N, 1], F32, kind="ExternalInput").ap()
    Cw = nc.dram_tensor("Cw", [N, DM], F16, kind="ExternalInput").ap()
    Dfl = nc.dram_tensor("Dfl", [128, DM], F32, kind="ExternalInput").ap()
    yout_d = nc.dram_tensor("y", [t_total, DM], F32, kind="ExternalOutput").ap()

    with tile.TileContext(nc) as tc:
        with ExitStack() as ctx:
            cpool = ctx.enter_context(tc.tile_pool(name="consts", bufs=1))
            wa_pool = ctx.enter_context(tc.tile_pool(name="wa", bufs=3))
            ut_pool = ctx.enter_context(tc.tile_pool(name="ut", bufs=2))
            big_pool = ctx.enter_context(tc.tile_pool(name="big", bufs=2))
            h_pool = ctx.enter_context(tc.tile_pool(name="h", bufs=1))
            d_pool = ctx.enter_context(tc.tile_pool(name="d", bufs=2))
            b_pool = ctx.enter_context(tc.tile_pool(name="bt", bufs=2))
            u16_pool = ctx.enter_context(tc.tile_pool(name="u16t", bufs=2))
            yo_pool = ctx.enter_context(tc.tile_pool(name="yo", bufs=2))
            ty_pool = ctx.enter_context(tc.tile_pool(name="ty", bufs=2))
            dh_pool = ctx.enter_context(tc.tile_pool(name="dhb", bufs=3))
            psg = ctx.enter_context(tc.tile_pool(name="psg", bufs=2, space="PSUM"))
            pss = ctx.enter_context(tc.tile_pool(name="pss", bufs=2, space="PSUM"))
            psp = ctx.enter_context(tc.tile_pool(name="psp", bufs=2, space="PSUM"))

            # ---- constants ----
            wd_sb = cpool.tile([128, KT * N], F16)
            nc.sync.dma_start(wd_sb[:], Wdh)
            wb_sb = cpool.tile([128, KT * N], F16)
            nc.sync.dma_start(wb_sb[:], WBh)
            bab_sb = cpool.tile([N, N], F32)
            nc.sync.dma_start(bab_sb[:], bAb)
            bd_sb = cpool.tile([N, 1], F32)
            nc.sync.dma_start(bd_sb[:], bdv)
            bb_sb = cpool.tile([N, 1], F32)
            nc.sync.dma_start(bb_sb[:], bBv)
            c_sb = cpool.tile([N, DM], F16)
            nc.sync.dma_start(c_sb[:], Cw)
            dfl_sb = cpool.tile([128, DM], F32)
            nc.sync.dma_start(dfl_sb[:], Dfl)

            h_sb = h_pool.tile([128, t_total + 8], F16)
            nc.vector.memset(h_sb[:, 0:1], 0.0)

            bigs = [None, None]
            dsbs = [None, None]
            bsbs = [None, None]

            for c in range(nchunks + 1):
                cw = chunks[c] if c < nchunks else 0
                pw = chunks[c - 1] if c >= 1 else 0   # scan-chunk width
                po = offs[c - 1] if c >= 1 else 0     # scan-chunk offset
                if c < nchunks:
                    t0 = offs[c]
                    ut = ut_pool.tile([128, KT, cw], F16, tag="ut")
                    for k in range(KT):
                        nc.sync.dma_start(
                            ut[:, k, :], uT[k * 128 : (k + 1) * 128, t0 : t0 + cw]
                        )
                    # d = sigmoid(u Wd + bd)
                    pd = pss.tile([128, 512], F32, tag="small")
                    for k in range(KT):
                        nc.tensor.matmul(
                            pd[:, :cw],
                            wd_sb[:, k * N : (k + 1) * N],
                            ut[:, k, :],
                            start=(k == 0),
                            stop=(k == KT - 1),
                        )
                    dsb = d_pool.tile([N, cw], F32, tag="dsb")
                    nc.scalar.activation(
                        dsb[:], pd[:, :cw], AFT.Sigmoid, bias=bd_sb[:, 0:1]
                    )
                    dsbs[c % 2] = dsb
                    # Bt = u WB + bB
                    pb = pss.tile([128, 512], F32, tag="small")
                    for k in range(KT):
                        nc.tensor.matmul(
                            pb[:, :cw],
                            wb_sb[:, k * N : (k + 1) * N],
                            ut[:, k, :],
                            start=(k == 0),
                            stop=(k == KT - 1),
                        )
                    bsb = b_pool.tile([N, cw], F32, tag="bsb")
                    nc.scalar.activation(
                        bsb[:], pb[:, :cw], AFT.Identity, bias=bb_sb[:, 0:1]
                    )
                    bsbs[c % 2] = bsb

                    # s-major bigbuf: bigs[m, s, t] = A_t[s, m] (zero diag)
                    bigbuf = big_pool.tile([128, N, cw], F16, tag="bigbuf")
                    bigs[c % 2] = bigbuf

                def scan_step(tl):
                    """One scan timestep of chunk c-1.  dhb = d*h + B fires on
                    the same DVE queue right before the update; it only needs
                    h_{t-1}, so it executes inside the chain's DVE idle window
                    while the PE matvec drains.  Chain: MM -> update -> MM."""
                    prev = (c - 1) % 2
                    tg = po + tl
                    dhb = dh_pool.tile([128, 1], F32)
                    pp = psp.tile([128, 1], F32)
                    nc.vector.tensor_scalar(
                        dhb[:],
                        h_sb[:, tg : tg + 1],
                        dsbs[prev][:, tl : tl + 1],
                        bsbs[prev][:, tl : tl + 1],
                        mybir.AluOpType.mult,
                        mybir.AluOpType.add,
                    )
                    nc.tensor.matmul(
                        pp[:],
                        bigs[prev][:, :, tl : tl + 1],
                        h_sb[:, tg : tg + 1],
                        start=True,
                        stop=True,
                    )
                    nc.vector.tensor_scalar(
                        h_sb[:, tg + 1 : tg + 2],
                        pp[:],
                        ISN,
                        dhb[:, 0:1],
                        mybir.AluOpType.mult,
                        mybir.AluOpType.add,
                    )

                emitted = 0
                y_done = 0

                def emit_scan_to(target):
                    nonlocal emitted
                    while emitted < target:
                        scan_step(emitted)
                        emitted += 1

                def emit_y_ready():
                    # emit y-blocks of chunk c-1 as soon as their scan steps
                    # are complete
                    nonlocal y_done
                    while y_done < pw and y_done + min(128, pw - y_done) <= emitted:
                        tw = min(128, pw - y_done)
                        tstart = po + y_done
                        y_done += tw
                        for dh in range(DM // 512):
                            py = pss.tile([128, 512], F32, tag="small")
                            nc.tensor.matmul(
                                py[:tw, :],
                                h_sb[:, 1 + tstart : 1 + tstart + tw],
                                c_sb[:, dh * 512 : (dh + 1) * 512],
                                start=True,
                                stop=True,
                            )
                            u16t = u16_pool.tile([128, 512], F16)
                            nc.sync.dma_start(
                                u16t[:tw, :],
                                u16[tstart : tstart + tw, dh * 512 : (dh + 1) * 512],
                            )
                            tyt = ty_pool.tile([128, 512], F32)
                            nc.vector.tensor_mul(
                                tyt[:tw, :], u16t[:tw, :],
                                dfl_sb[:tw, dh * 512 : (dh + 1) * 512],
                            )
                            yo = yo_pool.tile([128, 512], F32)
                            nc.vector.tensor_add(yo[:tw, :], py[:tw, :], tyt[:tw, :])
                            nc.sync.dma_start(
                                yout_d[tstart : tstart + tw, dh * 512 : (dh + 1) * 512],
                                yo[:tw, :],
                            )

                WB_BATCH = 4  # slices per WA DMA transfer (1 MiB each)
                for s in range(N):
                    if c < nchunks:
                        if s % WB_BATCH == 0:
                            wa = wa_pool.tile([128, WB_BATCH, KT * 128], F16)
                            nc.sync.dma_start(
                                wa[:],
                                WAh[s : s + WB_BATCH].rearrange("s p f -> p s f"),
                            )
                        pg = psg.tile([128, 512], F32, tag="pg")
                        for k in range(KT):
                            nc.tensor.matmul(
                                pg[:, :cw],
                                wa[:, s % WB_BATCH, k * 128 : (k + 1) * 128],
                                ut[:, k, :],
                                start=(k == 0),
                                stop=(k == KT - 1),
                            )
                            if c >= 1 and k in (1, 3, 5):
                                emit_scan_to(
                                    ((s * KT + k + 1) * pw) // (N * KT)
                                )
                        nc.scalar.activation(
                            bigs[c % 2][:, s, :], pg[:, :cw], AFT.Tanh,
                            bias=bab_sb[:, s : s + 1],
                        )
                        if c >= 1:
                            emit_scan_to(((s + 1) * pw) // N)
                            emit_y_ready()
                    elif c >= 1:
                        emit_scan_to(((s + 1) * pw) // N)
                        emit_y_ready()

                if c >= 1:
                    emit_y_ready()
                    assert y_done == pw and emitted == pw
    nc.compile()
    return nc


def prep_inputs(u_row, Wd, bd, WA, bA, WB, bB, C, D_skip, t_total=T):
    """Host-side packing of one batch row's inputs into the kernel layout."""
    f16 = np.float16
    idx = np.arange(N)
    WAz = np.array(WA, np.float32, copy=True)
    WAz[:, idx * N + idx] = 0.0
    bAz = np.array(bA, np.float32, copy=True)
    bAz[idx * N + idx] = 0.0
    # WAh[s, p, k*128+m] = WAz[k*128+p, s*N+m]
    WAhost = np.ascontiguousarray(
        WAz.reshape(KT, 128, N, N).transpose(2, 1, 0, 3).reshape(N, 128, KT * 128)
    ).astype(f16)
    Wdh = np.ascontiguousarray(
        np.asarray(Wd, np.float32).reshape(KT, 128, N).transpose(1, 0, 2).reshape(128, KT * N)
    ).astype(f16)
    WBh = np.ascontiguousarray(
        np.asarray(WB, np.float32).reshape(KT, 128, N).transpose(1, 0, 2).reshape(128, KT * N)
    ).astype(f16)
    return {
        "uT": np.ascontiguousarray(u_row.T).astype(f16),
        "u16": np.ascontiguousarray(u_row).astype(f16),
        "WAh": WAhost,
        "Wdh": Wdh,
        "WBh": WBh,
        "bAb": np.ascontiguousarray(bAz.reshape(N, N).T).astype(np.float32),
        "bdv": np.asarray(bd, np.float32).reshape(N, 1).copy(),
        "bBv": np.asarray(bB, np.float32).reshape(N, 1).copy(),
        "Cw": np.asarray(C, np.float32).astype(f16),
        "Dfl": np.ascontiguousarray(
            np.broadcast_to(np.asarray(D_skip, np.float32), (128, DM))
        ).copy(),
    }


_NC_CACHE = {}

# Each batch row r is handled by the core pair (r, r+4): core r covers
# t in [0, 1088), core r+4 covers t in [960, 2048).  Both run the same
# T_LOCAL=1088 program; core r+4's first 128 steps (scanned from h=0) are
# warm-up — the state contracts by ~0.9/step, so by local t=128 the state
# matches the true one to ~1e-6 and its outputs [128:1088) are valid.
T_LOCAL = 1088
CHUNKS_LOCAL = [272, 272, 272, 272]
SHIFT = T - T_LOCAL  # 960
SPLIT = T_LOCAL      # first core's valid range
WARM = 128


def make_in_maps(u, Wd, bd, WA, bA, WB, bB, C, D_skip):
    in_maps = []
    for core in range(8):
        r, half = core % B, core // B
        off = half * SHIFT
        in_maps.append(
            prep_inputs(
                u[r, off : off + T_LOCAL], Wd, bd, WA, bA, WB, bB, C, D_skip,
                t_total=T_LOCAL,
            )
        )
    return in_maps


def kernel(u, Wd, bd, WA, bA, WB, bB, C, D_skip):
    u = np.asarray(u, np.float32)
    if "nc" not in _NC_CACHE:
        _NC_CACHE["nc"] = build_nc(T_LOCAL, CHUNKS_LOCAL)
    nc = _NC_CACHE["nc"]

    in_maps = make_in_maps(u, Wd, bd, WA, bA, WB, bB, C, D_skip)
    res = run_bass_kernel_spmd(nc, in_maps, core_ids=list(range(8)))
    y = np.empty((B, T, DM), np.float32)
    for r in range(B):
        y[r, :SPLIT] = res.results[r]["y"][:SPLIT]
        y[r, SPLIT:] = res.results[r + B]["y"][SPLIT - SHIFT :]
    return y


# revision 24
# speedup vs baseline: 1.8494x; 1.2375x over previous
"""DenseSSM layer kernel for Trainium2 (8 NeuronCores, data-parallel over batch).

Reference computation per batch row r:
    d  = sigmoid(u @ Wd + bd)                      [T, N]
    A  = tanh(u @ WA + bA).reshape(T,N,N)/sqrt(N)  with diagonal replaced by d
    Bt = u @ WB + bB                               [T, N]
    h_t = A_t h_{t-1} + Bt_t   (sequential scan)
    y  = hs @ C + D_skip * u                       [T, DM]

Kernel strategy (per core; core i handles batch row i % 4, half the T range):
  - Big GEMM u@WA in fp16, WA stationary per (slice s, k-tile); PSUM [m, t]
    evacuated by ACT tanh (bias bA) into bigbuf[m, s, t].  WA/bA diag slots
    are zeroed on host so the stored A has zero diagonal.
  - Scan (the critical serial chain): per step, PE matvec p = BIGS_t^T h and
    DVE dhb = d*h + B (both fire when h_{t-1} lands; dhb executes during the
    matvec's drain window), then one DVE update h_t = p*ISN + dhb.  All scan
    vector work stays on DVE: ACT only runs tanh evacs, so no 500ns+ tanh op
    ever head-of-line-blocks a chain update (ACT/DVE queues are strict FIFO).
  - Scan steps for chunk c-1 interleave into chunk c's GEMM instruction
    stream; the whole kernel is a conveyor paced by the serial chain
    (~0.49us/step) with the GEMM (~0.43us/step of PE work) in its shadow.
  - y GEMM per 128-timestep block from h_sb against C (fp16) + D_skip*u on DVE.
  - WA streamed per chunk (4 chunks of 272 -> 128MB) on the sync HWDGE ring
    only: the scalar ring shares the ACT instruction queue and DMAs there
    head-of-line-block the tanh evacs.
"""

import sys

sys.path.insert(0, "/opt/trn_rl_repo")

import numpy as np
from contextlib import ExitStack

import concourse.bass as bass
import concourse.tile as tile
from concourse import bacc, mybir
from concourse.bass_utils import run_bass_kernel_spmd

F16 = mybir.dt.float16
F32 = mybir.dt.float32
AFT = mybir.ActivationFunctionType

B, T, DM, N = 4, 2048, 1024, 128
KT = DM // 128          # 8 contraction tiles
SQN = float(np.sqrt(N))
ISN = float(1.0 / np.sqrt(N))


def build_nc(t_total, chunks):
    assert sum(chunks) == t_total
    nchunks = len(chunks)
    offs = [0]
    for w in chunks:
        offs.append(offs[-1] + w)
    nc = bacc.Bacc("TRN2", debug=False)

    uT = nc.dram_tensor("uT", [DM, t_total], F16, kind="ExternalInput").ap()
    u16 = nc.dram_tensor("u16", [t_total, DM], F16, kind="ExternalInput").ap()
    WAh = nc.dram_tensor("WAh", [128, N, KT * 128], F16, kind="ExternalInput").ap()
    Wdh = nc.dram_tensor("Wdh", [128, KT * N], F16, kind="ExternalInput").ap()
    WBh = nc.dram_tensor("WBh", [128, KT * N], F16, kind="ExternalInput").ap()
    bAb = nc.dram_tensor("bAb", [N, N], F32, kind="ExternalInput").ap()
    bdv = nc.dram_tensor("bdv", [N, 1], F32, kind="ExternalInput").ap()
    bBv = nc.dram_tensor("bBv", [N, 1], F32, kind="ExternalInput").ap()
    Cw = nc.dram_tensor("Cw", [N, DM], F16, kind="ExternalInput").ap()
    yout_d = nc.dram_tensor("y", [t_total, DM], F32, kind="ExternalOutput").ap()

    with tile.TileContext(nc) as tc:
        with ExitStack() as ctx:
            cpool = ctx.enter_context(tc.tile_pool(name="consts", bufs=1))
            wa_pool = ctx.enter_context(tc.tile_pool(name="wa", bufs=3))
            ut_pool = ctx.enter_context(tc.tile_pool(name="ut", bufs=2))
            big_pool = ctx.enter_context(tc.tile_pool(name="big", bufs=2))
            h_pool = ctx.enter_context(tc.tile_pool(name="h", bufs=1))
            d_pool = ctx.enter_context(tc.tile_pool(name="d", bufs=2))
            b_pool = ctx.enter_context(tc.tile_pool(name="bt", bufs=2))
            u16_pool = ctx.enter_context(tc.tile_pool(name="u16t", bufs=2))
            yo_pool = ctx.enter_context(tc.tile_pool(name="yo", bufs=2))
            psg = ctx.enter_context(tc.tile_pool(name="psg", bufs=2, space="PSUM"))
            pss = ctx.enter_context(tc.tile_pool(name="pss", bufs=2, space="PSUM"))
            psp = ctx.enter_context(tc.tile_pool(name="psp", bufs=2, space="PSUM"))

            # ---- constants ----
            wd_sb = cpool.tile([128, KT * N], F16)
            nc.sync.dma_start(wd_sb[:], Wdh)
            wb_sb = cpool.tile([128, KT * N], F16)
            nc.sync.dma_start(wb_sb[:], WBh)
            bab_sb = cpool.tile([N, N], F32)
            nc.sync.dma_start(bab_sb[:], bAb)
            bd_sb = cpool.tile([N, 1], F32)
            nc.sync.dma_start(bd_sb[:], bdv)
            bb_sb = cpool.tile([N, 1], F32)
            nc.sync.dma_start(bb_sb[:], bBv)
            c_sb = cpool.tile([N, DM], F16)
            nc.sync.dma_start(c_sb[:], Cw)

            h_sb = h_pool.tile([128, t_total + 8], F16)
            nc.vector.memset(h_sb[:, 0:1], 0.0)

            bigs = [None, None]
            dsbs = [None, None]
            bsbs = [None, None]

            for c in range(nchunks + 1):
                cw = chunks[c] if c < nchunks else 0
                pw = chunks[c - 1] if c >= 1 else 0   # scan-chunk width
                po = offs[c - 1] if c >= 1 else 0     # scan-chunk offset
                if c < nchunks:
                    t0 = offs[c]
                    ut = ut_pool.tile([128, KT, cw], F16, tag="ut")
                    for k in range(KT):
                        nc.sync.dma_start(
                            ut[:, k, :], uT[k * 128 : (k + 1) * 128, t0 : t0 + cw]
                        )
                    # d = sigmoid(u Wd + bd)
                    pd = pss.tile([128, 512], F32, tag="small")
                    for k in range(KT):
                        nc.tensor.matmul(
                            pd[:, :cw],
                            wd_sb[:, k * N : (k + 1) * N],
                            ut[:, k, :],
                            start=(k == 0),
                            stop=(k == KT - 1),
                        )
                    dsb = d_pool.tile([N, cw], F32, tag="dsb")
                    nc.scalar.activation(
                        dsb[:], pd[:, :cw], AFT.Sigmoid, bias=bd_sb[:, 0:1]
                    )
                    dsbs[c % 2] = dsb
                    # Bt = u WB + bB
                    pb = pss.tile([128, 512], F32, tag="small")
                    for k in range(KT):
                        nc.tensor.matmul(
                            pb[:, :cw],
                            wb_sb[:, k * N : (k + 1) * N],
                            ut[:, k, :],
                            start=(k == 0),
                            stop=(k == KT - 1),
                        )
                    bsb = b_pool.tile([N, cw], F32, tag="bsb")
                    nc.scalar.activation(
                        bsb[:], pb[:, :cw], AFT.Identity, bias=bb_sb[:, 0:1]
                    )
                    bsbs[c % 2] = bsb
                    # pre-scaled fp16 diag source: sqrt(N)*d (so the scan
                    # matvec's diag slot times h and the final *ISN gives d*h)
                    dsc = d_pool.tile([N, cw], F16, tag="dsc")
                    nc.vector.tensor_scalar(
                        dsc[:], dsb[:], SQN, None, mybir.AluOpType.mult
                    )
                    dscs = dsc

                    # s-major bigbuf: bigs[m, s, t] = A_t[s, m]
                    bigbuf = big_pool.tile([128, N, cw], F16, tag="bigbuf")
                    bigs[c % 2] = bigbuf

                def scan_step(tl):
                    """One scan timestep of chunk c-1: PE matvec (diagonal d*h
                    rides inside it via the diag-injected bigbuf) plus a single
                    DVE update h_t = pp*ISN + B_t.  One DVE op per step keeps
                    the serial chain at ~0.49us; a second DVE op would
                    serialize on the queue and cost ~+0.17us/step."""
                    prev = (c - 1) % 2
                    tg = po + tl
                    pp = psp.tile([128, 1], F32)
                    nc.tensor.matmul(
                        pp[:],
                        bigs[prev][:, :, tl : tl + 1],
                        h_sb[:, tg : tg + 1],
                        start=True,
                        stop=True,
                    )
                    nc.vector.tensor_scalar(
                        h_sb[:, tg + 1 : tg + 2],
                        pp[:],
                        ISN,
                        bsbs[prev][:, tl : tl + 1],
                        mybir.AluOpType.mult,
                        mybir.AluOpType.add,
                    )

                emitted = 0
                y_done = 0

                def emit_scan_to(target):
                    nonlocal emitted
                    while emitted < target:
                        scan_step(emitted)
                        emitted += 1

                def emit_y_ready():
                    # emit y-blocks of chunk c-1 as soon as their scan steps
                    # are complete
                    nonlocal y_done
                    while y_done < pw and y_done + min(128, pw - y_done) <= emitted:
                        tw = min(128, pw - y_done)
                        tstart = po + y_done
                        y_done += tw
                        for dh in range(DM // 512):
                            py = pss.tile([128, 512], F32, tag="small")
                            nc.tensor.matmul(
                                py[:tw, :],
                                h_sb[:, 1 + tstart : 1 + tstart + tw],
                                c_sb[:, dh * 512 : (dh + 1) * 512],
                                start=True,
                                stop=True,
                            )
                            u16t = u16_pool.tile([128, 512], F16)
                            nc.sync.dma_start(
                                u16t[:tw, :],
                                u16[tstart : tstart + tw, dh * 512 : (dh + 1) * 512],
                            )
                            yo = yo_pool.tile([128, 512], F32)
                            nc.vector.tensor_add(yo[:tw, :], py[:tw, :], u16t[:tw, :])
                            nc.sync.dma_start(
                                yout_d[tstart : tstart + tw, dh * 512 : (dh + 1) * 512],
                                yo[:tw, :],
                            )

                WB_BATCH = 4   # slices per WA DMA transfer (1 MiB each)
                DIAG_LAG = 12  # diag DMAs trail WA batches by 3 batches so
                               # they never head-of-line-block WA prefetch on
                               # the sync ring (ring is FIFO; a diag waits on
                               # its slice's ACT evac)

                def emit_diag(s):
                    # diag injection: row s of slice s <- sqrt(N)*d[s, :]
                    # (single-descriptor SBUF->SBUF DMA; compute engines
                    # cannot address a lone partition at arbitrary base)
                    nc.sync.dma_start(
                        bigs[c % 2][s : s + 1, s, :],
                        dscs[s : s + 1, :],
                    )

                for s in range(N):
                    if c < nchunks:
                        if s % WB_BATCH == 0:
                            wa = wa_pool.tile([128, WB_BATCH, KT * 128], F16)
                            # WA rides the (otherwise idle) gpsimd SWDGE queue
                            # so the sync ring's diag DMAs can never convoy it
                            nc.gpsimd.dma_start(
                                wa[:], WAh[:, s : s + WB_BATCH, :]
                            )
                            for sd in range(max(0, s - DIAG_LAG - WB_BATCH),
                                            max(0, s - DIAG_LAG)):
                                emit_diag(sd)
                        pg = psg.tile([128, 512], F32, tag="pg")
                        for k in range(KT):
                            nc.tensor.matmul(
                                pg[:, :cw],
                                wa[:, s % WB_BATCH, k * 128 : (k + 1) * 128],
                                ut[:, k, :],
                                start=(k == 0),
                                stop=(k == KT - 1),
                            )
                            if c >= 1 and k in (1, 3, 5):
                                emit_scan_to(
                                    ((s * KT + k + 1) * pw) // (N * KT)
                                )
                        nc.scalar.activation(
                            bigs[c % 2][:, s, :], pg[:, :cw], AFT.Tanh,
                            bias=bab_sb[:, s : s + 1],
                        )
                        if c >= 1:
                            emit_scan_to(((s + 1) * pw) // N)
                            emit_y_ready()
                    elif c >= 1:
                        emit_scan_to(((s + 1) * pw) // N)
                        emit_y_ready()
                if c < nchunks:
                    for sd in range(max(0, N - DIAG_LAG - WB_BATCH), N):
                        emit_diag(sd)

                if c >= 1:
                    emit_y_ready()
                    assert y_done == pw and emitted == pw
    nc.compile()
    return nc


def prep_inputs(u_row, Wd, bd, WA, bA, WB, bB, C, D_skip, t_total=T):
    """Host-side packing of one batch row's inputs into the kernel layout."""
    f16 = np.float16
    idx = np.arange(N)
    WAz = np.array(WA, np.float32, copy=True)
    WAz[:, idx * N + idx] = 0.0
    bAz = np.array(bA, np.float32, copy=True)
    bAz[idx * N + idx] = 0.0
    # WAh[p, s, k*128+m] = WAz[k*128+p, s*N+m]  (partition-major so the
    # per-batch DMA reads contiguous 8KB runs per partition)
    WAhost = np.ascontiguousarray(
        WAz.reshape(KT, 128, N, N).transpose(1, 2, 0, 3).reshape(128, N, KT * 128)
    ).astype(f16)
    Wdh = np.ascontiguousarray(
        np.asarray(Wd, np.float32).reshape(KT, 128, N).transpose(1, 0, 2).reshape(128, KT * N)
    ).astype(f16)
    WBh = np.ascontiguousarray(
        np.asarray(WB, np.float32).reshape(KT, 128, N).transpose(1, 0, 2).reshape(128, KT * N)
    ).astype(f16)
    return {
        "uT": np.ascontiguousarray(u_row.T).astype(f16),
        # residual D_skip*u precomputed on host; kernel just adds it to hs@C
        "u16": np.ascontiguousarray(
            u_row * np.asarray(D_skip, np.float32)
        ).astype(f16),
        "WAh": WAhost,
        "Wdh": Wdh,
        "WBh": WBh,
        "bAb": np.ascontiguousarray(bAz.reshape(N, N).T).astype(np.float32),
        "bdv": np.asarray(bd, np.float32).reshape(N, 1).copy(),
        "bBv": np.asarray(bB, np.float32).reshape(N, 1).copy(),
        "Cw": np.asarray(C, np.float32).astype(f16),
    }


_NC_CACHE = {}

# Each batch row r is handled by the core pair (r, r+4): core r covers
# t in [0, 1088), core r+4 covers t in [960, 2048).  Both run the same
# T_LOCAL=1088 program; core r+4's first 128 steps (scanned from h=0) are
# warm-up — the state contracts by ~0.9/step, so by local t=128 the state
# matches the true one to ~1e-6 and its outputs [128:1088) are valid.
T_LOCAL = 1088
CHUNKS_LOCAL = [272, 272, 272, 272]
SHIFT = T - T_LOCAL  # 960
SPLIT = T_LOCAL      # first core's valid range
WARM = 128


def make_in_maps(u, Wd, bd, WA, bA, WB, bB, C, D_skip):
    in_maps = []
    for core in range(8):
        r, half = core % B, core // B
        off = half * SHIFT
        in_maps.append(
            prep_inputs(
                u[r, off : off + T_LOCAL], Wd, bd, WA, bA, WB, bB, C, D_skip,
                t_total=T_LOCAL,
            )
        )
    return in_maps


def kernel(u, Wd, bd, WA, bA, WB, bB, C, D_skip):
    u = np.asarray(u, np.float32)
    if "nc" not in _NC_CACHE:
        _NC_CACHE["nc"] = build_nc(T_LOCAL, CHUNKS_LOCAL)
    nc = _NC_CACHE["nc"]

    in_maps = make_in_maps(u, Wd, bd, WA, bA, WB, bB, C, D_skip)
    res = run_bass_kernel_spmd(nc, in_maps, core_ids=list(range(8)))
    y = np.empty((B, T, DM), np.float32)
    for r in range(B):
        y[r, :SPLIT] = res.results[r]["y"][:SPLIT]
        y[r, SPLIT:] = res.results[r + B]["y"][SPLIT - SHIFT :]
    return y
